# revision 1
# baseline (speedup 1.0000x reference)
"""Trainium2 Bass kernel for multi-head causal attention.

Problem: B=2, S=2048, D=1024, H=16 heads (head_dim=64), fp32.
  q,k,v = x@Wq, x@Wk, x@Wv  (per-head split)
  scores = q@k^T, causal mask, softmax(scores/sqrt(64))
  out = (attn@v concat) @ Wo + bo

Sharding (8 cores): core c -> batch b=c//4, head group g=c%4 (4 heads).
Each core computes its 4 heads' attention plus the partial output
projection (row-parallel Wo); host sums 4 partials per batch and adds bo.

Layout strategy (zero on-device transposes):
 - x^T passed host-transposed (feature-major).
 - Q^T,K^T produced feature-major: (head_dim x tokens), two heads stacked
   per 128-partition tile; scores^T computed per 64-partition row group.
 - Both heads' scores^T tiles (k x q) land in one 2-bank PSUM tile so the
   causal mask memset + exp run as single wide instructions. The exp'd
   bf16 tile is directly the PV stationary operand. V is token-major with
   an appended ones-column so the PV matmul also emits the softmax
   denominators.
 - stage-and-release normalization: accumulators are copied to SBUF the
   moment a pair finishes (freeing PSUM banks in ~1us); the fp32 chain
   (partition-0 bounce -> fast reciprocal -> gpsimd partition broadcast
   -> DVE multiply) then runs off the critical path. The tail-critical
   final pair instead uses a 1-deep PE row-broadcast matmul (the PE is
   idle there) for minimum latency.
 - software-pipelined emission: projections for range r+1 are emitted
   before attention(r), scores run two j ahead of PV (so a late exp
   never head-of-line-blocks the in-order PE queue), and the output
   projection runs one range behind.
 - causal-diagonal trim: scores/exp/PV widths shrink to the valid
   [off:512] q-columns of diagonal k-chunks.
 - few-descriptor DMA: x is passed host-rearranged range-major so each
   per-range load is 128 descriptors x 8KB; output is written bf16
   (host upcasts and sums partials in fp32); the exp ACT table is
   preloaded during the DMA preamble; 10 warmup matmuls on constant
   data bridge the DVFS clock ramp while the first DMAs land.
Matmul operands are bf16 (1 cycle/row PE rate); every accumulation and
the softmax normalization stay fp32 in PSUM.
"""

import sys

sys.path.insert(0, "/opt/trn_rl_repo")

import ml_dtypes
import numpy as np

import concourse.bass as bass  # noqa: F401
import concourse.tile as tile
from concourse import bacc, bass_utils, mybir

F32 = mybir.dt.float32
MMDT = mybir.dt.bfloat16
NPDT = ml_dtypes.bfloat16
EXPF = mybir.ActivationFunctionType.Exp
LNF = mybir.ActivationFunctionType.Ln

B, S, D, H, HD = 2, 2048, 1024, 16, 64
N_CORES = 8
HPC = 4            # heads per core
GW = HPC * HD      # head-group width per core = 256
SCALE = 1.0 / np.sqrt(HD)
NEG = -1.0e30

_CACHE = {}
LAST_RESULTS = None


def _maybe_install_trace_hook():
    """If BASS_TRACE is set, bass_utils needs antenv.axon_hooks (absent in
    this image). Install it from trn_boot when possible; otherwise disable
    tracing so the run still works."""
    import os

    if not os.environ.get("BASS_TRACE"):
        return
    try:
        import antenv.axon_hooks  # noqa: F401
        return
    except ImportError:
        pass
    try:
        import types

        from trn_agent_boot.trn_boot import _ntff_profile_via_ctypes

        hook = _ntff_profile_via_ctypes("/opt/axon/libaxon_pjrt.so")
        mod = types.ModuleType("antenv.axon_hooks")
        mod.get_axon_ntff_profile_hook = lambda: hook
        mod.set_axon_ntff_profile_hook = lambda h: None
        import antenv

        sys.modules["antenv.axon_hooks"] = mod
        antenv.axon_hooks = mod
    except Exception:
        os.environ["BASS_NEVER_TRACE"] = "1"


def _build():
    nc = bacc.Bacc("TRN2", target_bir_lowering=False, debug=False)

    # x is passed host-rearranged range-major: row 128*r+p holds the NC=8
    # contraction chunks for q-range r, each 512 tokens contiguous, so one
    # per-range DMA is 128 descriptors x 8KB.
    xT = nc.dram_tensor("xT", [(S // 512) * 128, (D // 128) * 512], MMDT,
                        kind="ExternalInput").ap()
    wq = nc.dram_tensor("wq", [128, D // 128 * GW], MMDT, kind="ExternalInput").ap()
    wk = nc.dram_tensor("wk", [128, D // 128 * GW], MMDT, kind="ExternalInput").ap()
    wv = nc.dram_tensor("wv", [128, D // 128 * GW], MMDT, kind="ExternalInput").ap()
    wo = nc.dram_tensor("wo", [128, GW // 128 * D], MMDT, kind="ExternalInput").ap()
    trid = nc.dram_tensor("tri", [128, 128], F32, kind="ExternalInput").ap()
    out = nc.dram_tensor("out", [S, D], MMDT, kind="ExternalOutput").ap()

    NT = S // 512          # 4 q/t ranges of 512
    NC = D // 128          # 8 contraction chunks for projections
    NJ = S // 128          # 16 k-chunks

    with tile.TileContext(nc) as tc, nc.allow_low_precision(reason="bf16 matmuls"):
        with (
            tc.tile_pool(name="const", bufs=1) as cpool,
            tc.tile_pool(name="xin", bufs=2) as xpool,
            tc.tile_pool(name="pt", bufs=8) as ppool,
            tc.tile_pool(name="small", bufs=6) as spool,
            tc.tile_pool(name="ost", bufs=6) as opool,
            tc.tile_pool(name="psum", bufs=1, space="PSUM") as psum,
        ):
            # ---- persistent tiles ----
            wq_sb = cpool.tile([128, NC, GW], MMDT)
            wk_sb = cpool.tile([128, NC, GW], MMDT)
            wv_sb = cpool.tile([128, NC, GW], MMDT)
            wo_sb = cpool.tile([128, 2, D], MMDT)

            QT = cpool.tile([128, 2, S], MMDT)   # [:, pair, t] feature-major
            KT = cpool.tile([128, 2, S], MMDT)
            Vt = cpool.tile([128, NJ, HPC * 65], MMDT)  # token-major + ones col
            ctxT = cpool.tile([128, 2, S], MMDT)

            # PE warmup: stream constant data through the tensor engine while
            # the first DMAs land, so the DVFS activity monitor ramps the PE
            # clock to max before real matmuls begin. One accumulation group
            # so no inter-instruction semaphores serialize it.
            warm_sb = cpool.tile([128, 512], MMDT, name="warm")
            nc.vector.memset(warm_sb[:], 0.125)
            warm_ps = psum.tile([128, 1024], F32, tag="mm", bufs=3)
            NWARM = 14
            for i in range(NWARM):
                nc.tensor.matmul(
                    warm_ps[:, 0:512], warm_sb[:, 0:128], warm_sb[:],
                    start=(i == 0), stop=(i == NWARM - 1),
                )
            # preload the Exp activation table during the DMA preamble so the
            # first real exp doesn't eat the lazy ACT_TABLE_LOAD.
            tbl = cpool.tile([1, 8], F32, name="tbl")
            nc.scalar.activation(tbl[:], warm_sb[0:1, 0:8], EXPF, scale=SCALE)

            # all-ones tile; row 64 is the stationary for the PE row-broadcast
            # of the softmax denominators (partition 64 -> partitions 0..63).
            ones128 = cpool.tile([128, 64], MMDT, name="ones128")
            nc.vector.memset(ones128[:], 1.0)

            # ones columns of V (col 64 of each 65-wide head slot)
            vt_ones = Vt[:, :, :].rearrange("p j (h u) -> p (j h) u", u=65)[:, :, 64:65]
            nc.vector.memset(vt_ones, 1.0)

            # triangular causal mask for the diagonal 128-block of scores^T:
            # keep (q - k >= 0) else -1e30   [partition = k, free = q]
            # host-prepared so the gpsimd engine is never used at all.
            tri = cpool.tile([128, 128], F32, name="tri")
            nc.sync.dma_start(tri[:], trid)

            # broadcast view of tri over the two stacked heads (0-stride dim)
            tri_ap = tri[:]
            tri2 = bass.AP(
                tensor=tri_ap.tensor,
                offset=tri_ap.offset,
                ap=[list(tri_ap.ap[0]), [0, 2], list(tri_ap.ap[1])],
            )

            def load_xt(r):
                xt = xpool.tile([128, NC, 512], MMDT, tag="xt")
                xv = xT[128 * r : 128 * (r + 1), :].rearrange(
                    "p (c t) -> p c t", t=512
                )
                nc.sync.dma_start(xt[:], xv)
                xts[r] = xt

            xts = {}

            def qk_chain(r, w_sb, dst, o):
                def go():
                    pm = psum.tile([128, 1024], F32, tag="mm", bufs=3)
                    for c in range(NC):
                        nc.tensor.matmul(
                            pm[:, 0:512],
                            w_sb[:, c, 128 * o : 128 * (o + 1)],
                            xts[r][:, c, :],
                            start=(c == 0),
                            stop=(c == NC - 1),
                        )
                    nc.vector.tensor_copy(
                        dst[:, o, 512 * r : 512 * (r + 1)], pm[:, 0:512]
                    )
                return go

            def v_chain(r, tt):
                def go():
                    j = 4 * r + tt
                    pv = psum.tile([128, 1024], F32, tag="mm", bufs=3)
                    for c in range(NC):
                        nc.tensor.matmul(
                            pv[:, 0:GW],
                            xts[r][:, c, 128 * tt : 128 * (tt + 1)],
                            wv_sb[:, c, :],
                            start=(c == 0),
                            stop=(c == NC - 1),
                        )
                    nc.vector.tensor_copy(
                        Vt[:, j, :].rearrange("p (h u) -> p h u", u=65)[:, :, 0:64],
                        pv[:, 0:GW].rearrange("p (h d) -> p h d", d=HD),
                    )
                return go

            def wo_chain(r, qq, o):
                def go():
                    qt = 4 * r + qq
                    po = psum.tile([128, 1024], F32, tag="mm", bufs=3)
                    for d in range(2):
                        nc.tensor.matmul(
                            po[:, 0:512],
                            ctxT[:, d, 128 * qt : 128 * (qt + 1)],
                            wo_sb[:, d, 512 * o : 512 * (o + 1)],
                            start=(d == 0), stop=(d == 1),
                        )
                    if o == 0:
                        ots[qt] = opool.tile(
                            [128, 1024], MMDT, tag="ot", name=f"ot{qt}"
                        )
                    ot = ots[qt]
                    dst = ot[:, 512 * o : 512 * (o + 1)]
                    if o == 0:
                        nc.scalar.copy(dst, po[:, 0:512])
                    else:
                        nc.vector.tensor_copy(dst, po[:, 0:512])
                    if r == NT - 1:
                        # last range: per-half DMAs so the final transfer is
                        # small and the o=0 half drains during the o=1 copy
                        nc.sync.dma_start(
                            out[128 * qt : 128 * (qt + 1),
                                512 * o : 512 * (o + 1)],
                            ot[:, 512 * o : 512 * (o + 1)],
                        )
                    elif o == 1:
                        nc.sync.dma_start(
                            out[128 * qt : 128 * (qt + 1), :], ot[:]
                        )
                return go

            ots = {}

            def a_chains(r):
                ch = []
                for w_sb, dst in ((wq_sb, QT), (wk_sb, KT)):
                    for o in range(2):
                        ch.append(qk_chain(r, w_sb, dst, o))
                for tt in range(4):
                    ch.append(v_chain(r, tt))
                return ch

            def c_chains(r):
                return [wo_chain(r, qq, o) for qq in range(4) for o in range(2)]

            def attention(r, tail_chains=(), mid_chains=()):
                # tail_chains: the last projection chains of range r+1,
                # emitted right after this range's first scores so their PE
                # time covers the first exp's latency (the in-order PE queue
                # otherwise idles ~1.2us waiting for softmax at range start).
                # mid_chains: two wo chains of range r-1 emitted after pair
                # 1's first scores, covering the pair-transition exp latency
                # the same way.
                for p in range(2):
                    hA, hB = 2 * p, 2 * p + 1
                    nj = 4 * r + 4
                    ca = psum.tile([65, 512], F32, tag="acc", bufs=2)
                    cb = psum.tile([65, 512], F32, tag="acc", bufs=2)
                    def scores(j):
                        # QK^T for both heads of the pair; the two 64-row
                        # matmuls run concurrently on disjoint PE row halves.
                        v = j - 4 * r
                        off = 128 * v if v > 0 else 0   # q cols < off invalid
                        s2 = psum.tile([128, 1024], F32, tag="mm", bufs=3)
                        nc.tensor.matmul(
                            s2[:, off:512],
                            KT[0:64, p, 128 * j : 128 * (j + 1)],
                            QT[0:64, p, 512 * r + off : 512 * (r + 1)],
                            start=True, stop=True,
                        )
                        nc.tensor.matmul(
                            s2[:, 512 + off : 1024],
                            KT[64:128, p, 128 * j : 128 * (j + 1)],
                            QT[64:128, p, 512 * r + off : 512 * (r + 1)],
                            start=True, stop=True,
                        )
                        pt2 = ppool.tile([128, 1024], MMDT, tag="pt")
                        s2v = s2[:, :].rearrange("p (s q) -> p s q", s=2)
                        pt2v = pt2[:, :].rearrange("p (s q) -> p s q", s=2)
                        if v >= 0:      # diagonal block inside this q-range
                            nc.vector.tensor_add(
                                s2v[:, :, off : off + 128],
                                s2v[:, :, off : off + 128],
                                tri2,
                            )
                        nc.scalar.activation(
                            pt2v[:, :, off:512], s2v[:, :, off:512],
                            EXPF, scale=SCALE,
                        )
                        return pt2, off

                    def pv(j, pt2, off):
                        nc.tensor.matmul(
                            ca[:, off:512],
                            Vt[:, j, 65 * hA : 65 * hA + 65],
                            pt2[:, off:512],
                            start=(j == 0), stop=(j == nj - 1),
                        )
                        nc.tensor.matmul(
                            cb[:, off:512],
                            Vt[:, j, 65 * hB : 65 * hB + 65],
                            pt2[:, 512 + off : 1024],
                            start=(j == 0), stop=(j == nj - 1),
                        )

                    # software-pipelined: scores run two j ahead of PV so a
                    # late exp never head-of-line-blocks the PE queue.
                    LOOK = min(2, nj - 1)
                    pend = [scores(j) for j in range(LOOK)]
                    for ch in (tail_chains if p == 0 else mid_chains):
                        ch()
                    for j in range(LOOK, nj):
                        pend.append(scores(j))
                        pv(j - LOOK, *pend.pop(0))
                    for k, pd in enumerate(pend):
                        pv(nj - len(pend) + k, *pd)
                    # stage accumulators to SBUF immediately (frees the PSUM
                    # banks in ~1us); normalization then runs off the critical
                    # path entirely from SBUF.
                    stA = spool.tile([65, 512], F32, tag="st")
                    stB = spool.tile([65, 512], F32, tag="st")
                    nc.vector.tensor_copy(stA[:], ca[:])
                    nc.vector.tensor_copy(stB[:], cb[:])
                    if r == NT - 1 and p == 1:
                        # tail-critical pair: minimum-latency chain using a
                        # 1-deep PE row-broadcast of the denominator row
                        # (the PE is idle here anyway), then approx-fast
                        # reciprocal on the base-0 broadcast block.
                        dnA = spool.tile([128, 512], MMDT, tag="dn")
                        dnB = spool.tile([128, 512], MMDT, tag="dn")
                        nc.scalar.copy(dnA[64:65, :], ca[64:65, :])
                        nc.scalar.copy(dnB[64:65, :], cb[64:65, :])
                        bsA = psum.tile([64, 512], F32, tag="acc", bufs=2)
                        bsB = psum.tile([64, 512], F32, tag="acc", bufs=2)
                        nc.tensor.matmul(
                            bsA[:], ones128[64:65, :], dnA[64:65, :],
                            start=True, stop=True,
                        )
                        nc.tensor.matmul(
                            bsB[:], ones128[64:65, :], dnB[64:65, :],
                            start=True, stop=True,
                        )
                        ra = spool.tile([64, 512], F32, tag="rc")
                        rb = spool.tile([64, 512], F32, tag="rc")
                        nc.vector.reciprocal_approx_fast(ra[:], bsA[:])
                        nc.vector.reciprocal_approx_fast(rb[:], bsB[:])
                    else:
                        # off the critical path: bounce the denominator row
                        # to partition 0, reciprocal there, and broadcast on
                        # the otherwise-idle gpsimd engine - zero PE cost.
                        srA = spool.tile([1, 512], F32, tag="sw")
                        srB = spool.tile([1, 512], F32, tag="sw")
                        nc.sync.dma_start(srA[:], stA[64:65, :])
                        nc.sync.dma_start(srB[:], stB[64:65, :])
                        r1 = spool.tile([1, 512], F32, tag="r1")
                        r2 = spool.tile([1, 512], F32, tag="r1")
                        nc.vector.reciprocal_approx_fast(r1[:], srA[:])
                        nc.vector.reciprocal_approx_fast(r2[:], srB[:])
                        ra = spool.tile([64, 512], F32, tag="rc")
                        rb = spool.tile([64, 512], F32, tag="rc")
                        nc.gpsimd.partition_broadcast(ra[:], r1[:])
                        nc.gpsimd.partition_broadcast(rb[:], r2[:])
                    qs = slice(512 * r, 512 * (r + 1))
                    nc.vector.tensor_mul(ctxT[0:64, p, qs], stA[0:64, :], ra[:])
                    nc.vector.tensor_mul(ctxT[64:128, p, qs], stB[0:64, :], rb[:])

            # startup order: critical-path pieces first so the first Q chain
            # starts streaming as early as possible.
            wqv = wq.rearrange("p (c o) -> p c o", o=GW)
            nc.sync.dma_start(wq_sb[:], wqv)
            load_xt(0)
            nc.sync.dma_start(wk_sb[:], wk.rearrange("p (c o) -> p c o", o=GW))
            nc.sync.dma_start(wv_sb[:], wv.rearrange("p (c o) -> p c o", o=GW))
            load_xt(1)
            nc.sync.dma_start(wo_sb[:], wo.rearrange("p (c o) -> p c o", o=D))
            for ch in a_chains(0):
                ch()
            for r in range(NT):
                if r + 2 < NT:
                    load_xt(r + 2)
                chains = a_chains(r + 1) if r + 1 < NT else []
                wo_prev = c_chains(r - 1) if r > 0 else []
                for ch in chains[:-2]:
                    ch()
                attention(r, chains[-2:], wo_prev[:2])
                for ch in wo_prev[2:]:
                    ch()
            for ch in c_chains(NT - 1):
                ch()

    nc.compile()
    return nc


def _get_nc():
    if "nc" not in _CACHE:
        _CACHE["nc"] = _build()
    return _CACHE["nc"]


def kernel(x, Wq, Wk, Wv, Wo, bo):
    global LAST_RESULTS
    x = np.asarray(x, dtype=np.float32)
    Wq = np.asarray(Wq, dtype=np.float32)
    Wk = np.asarray(Wk, dtype=np.float32)
    Wv = np.asarray(Wv, dtype=np.float32)
    Wo = np.asarray(Wo, dtype=np.float32)
    bo = np.asarray(bo, dtype=np.float32)

    nc = _get_nc()
    # range-major layout: [NT*128, NC*512]; row 128*r+p holds chunks c=0..7
    # (512 tokens each, contiguous) of q-range r for feature-row p.
    NT = S // 512
    NC = D // 128

    def xarr(b):
        a = x[b].T.reshape(NC, 128, NT, 512).transpose(2, 1, 0, 3)
        return np.ascontiguousarray(a.reshape(NT * 128, NC * 512)).astype(NPDT)

    xTs = [xarr(b) for b in range(B)]

    def warr(w, cs):
        # [D, GW] slice -> [128, NC*GW]: partition p holds chunk-major rows
        s = w[:, cs].reshape(D // 128, 128, GW).transpose(1, 0, 2)
        return np.ascontiguousarray(s.reshape(128, -1)).astype(NPDT)

    def woarr(cs):
        # [GW, D] slice -> [128, 2*D]
        s = Wo[cs, :].reshape(GW // 128, 128, D).transpose(1, 0, 2)
        return np.ascontiguousarray(s.reshape(128, -1)).astype(NPDT)

    # causal mask block: keep (q - k >= 0) else -1e30  [partition=k, free=q]
    ktri = np.arange(128)
    tri_np = np.where(ktri[None, :] - ktri[:, None] >= 0, 0.0, NEG).astype(
        np.float32
    )

    in_maps = []
    for c in range(N_CORES):
        b, g = divmod(c, N_CORES // B)
        cs = slice(GW * g, GW * (g + 1))
        in_maps.append(
            {
                "xT": xTs[b],
                "wq": warr(Wq, cs),
                "wk": warr(Wk, cs),
                "wv": warr(Wv, cs),
                "wo": woarr(cs),
                "tri": tri_np,
            }
        )

    _maybe_install_trace_hook()
    res = bass_utils.run_bass_kernel_spmd(nc, in_maps, core_ids=list(range(N_CORES)))
    LAST_RESULTS = res

    out = np.zeros((B, S, D), dtype=np.float32)
    for c in range(N_CORES):
        out[c // (N_CORES // B)] += res.results[c]["out"].astype(np.float32)
    out += bo[None, None, :]
    return out

